# revision 14
# baseline (speedup 1.0000x reference)
"""CARAFE content-aware upsampling kernel for Trainium2 (Bass/Tile), SPMD over 8 NeuronCores.

Problem (hardcoded):
  features: (4, 256, 64, 64) f32, masks: (4, 25, 128, 128) f32
  out[n,c,H,W] = sum_{dy,dx in 0..4} features[n, c, H//2+dy-2, W//2+dx-2] * masks[n, 5*dy+dx, H, W]
  (zero padding outside the feature map), output (4, 256, 128, 128) f32.

Sharding: 8 cores = 4 batch x 2 output-row halves. Each core computes out rows
[64*half, 64*half+64) for one batch element. No cross-core communication.

Device algorithm (per core):
  The einsum contracts over the 25 taps with per-pixel weights, which maps onto a
  dense matmul by contracting over *source pixels* of a tile instead:
    out[c, q] = sum_p featT[p, c] * W[p, q]
  q ranges over a tile of 128 output pixels (8 rows x 16 cols); p over the 96 source
  pixels (8 rows x 12 cols, incl. 2-halo) feeding that tile. W is the mask im2col:
  W[p, q] = masks[tap(p,q), q] if p is inside q's 5x5 window else 0 (25/96 dense).
  featT (px-major feature windows) and W are packed host-side into one fp16 tensor,
  so the device does big per-band DMAs plus dense [96,128]^T x [96,128] matmuls
  (fp32 PSUM accumulate), staged to fp16 and stored.

Per-core DRAM tensors:
  combo [8, 96, 3072] fp16  per band: seg-major [featT(256 c) | wim(128 q)] blocks
  out   [256, 64, 128] fp16 (upcast to f32 on host)
"""

import os
import sys

for _p in ("/opt/trn_rl_repo", os.path.expanduser("~/.axon_site/_ro/trn_rl_repo")):
    if os.path.isdir(_p) and _p not in sys.path:
        sys.path.insert(0, _p)

import numpy as np
from contextlib import ExitStack

import concourse.bass as bass
import concourse.tile as tile
from concourse import bacc, mybir
from concourse import bass_utils

N, C, HS, WS = 4, 256, 64, 64      # features shape
KK, SC = 5, 2                      # kernel size, upsample scale
HO, WO = HS * SC, WS * SC          # output 128 x 128
NCORES = 8

BANDS = 8                          # output-row bands of 8 (64 out rows per core)
SEGS = 8                           # output-col segments of 16
KP = 96                            # contraction: 8 src rows x 12 src cols
QT = 128                           # out px per tile: 8 Hrel x 16 Wrel
SEGW = C + QT                      # 384 packed cols per seg: featT | wim
F32 = mybir.dt.float32
F16 = mybir.dt.float16
NP16 = np.float16


def _build_w_im2col(mask_shard: np.ndarray) -> np.ndarray:
    """mask_shard (25, 64, 128) -> W (BANDS, SEGS, KP, QT)."""
    m = mask_shard.reshape(25, BANDS, 8, SEGS, 16)          # i, band, Hr, seg, Wr
    w = np.zeros((BANDS, SEGS, KP, 8, 16), dtype=NP16)
    hr = np.arange(8)[:, None]                              # (8, 1)
    wr = np.arange(16)[None, :]                             # (1, 16)
    h = hr // 2                                             # src row within band (0..3)
    ww = wr // 2                                            # src col within seg (0..7)
    for dy in range(KK):
        for dx in range(KK):
            kidx = (h + dy) * 12 + (ww + dx)                # (8, 16)
            w[:, :, kidx, hr, wr] = m[KK * dy + dx].transpose(0, 2, 1, 3).astype(NP16)
    return w.reshape(BANDS, SEGS, KP, QT)


def _build_featT(feat_shard_padded: np.ndarray) -> np.ndarray:
    """feat (256, 36, 68) padded slice -> featT (BANDS, SEGS, KP, C)."""
    sw = np.lib.stride_tricks.sliding_window_view(feat_shard_padded, (8, 12), axis=(1, 2))
    tiles = sw[:, ::4, ::8]                                  # (C, 8, 8, 8, 12)
    return tiles.transpose(1, 2, 3, 4, 0).reshape(BANDS, SEGS, KP, C).astype(NP16)


def _build_combo(feat_shard_padded: np.ndarray, mask_shard: np.ndarray) -> np.ndarray:
    ft = _build_featT(feat_shard_padded)                     # (B, S, KP, C)
    wm = _build_w_im2col(mask_shard)                         # (B, S, KP, QT)
    combo = np.concatenate([ft, wm], axis=3)                 # (B, S, KP, SEGW)
    # band-major, partition-major: [band, p, seg*SEGW + col]
    return np.ascontiguousarray(combo.transpose(0, 2, 1, 3).reshape(BANDS, KP, SEGS * SEGW))


def _carafe_body(ctx: ExitStack, tc: "tile.TileContext", out: bass.AP, combo: bass.AP) -> None:
    nc = tc.nc
    ld_pool = ctx.enter_context(tc.tile_pool(name="ld", bufs=5))
    stage_pool = ctx.enter_context(tc.tile_pool(name="stage", bufs=8))
    ps_mm = ctx.enter_context(tc.tile_pool(name="ps_mm", bufs=2, space="PSUM"))

    for band in range(BANDS):
        ld = ld_pool.tile([KP, SEGS * SEGW], F16)
        half = SEGS * SEGW // 2
        dma_eng = nc.sync if band % 2 == 0 else nc.scalar
        dma_eng2 = nc.scalar if band % 2 == 0 else nc.sync
        dma_eng.dma_start(ld[:, :half], combo[band][:, :half])
        dma_eng2.dma_start(ld[:, half:], combo[band][:, half:])
        mm = [ps_mm.tile([128, SEGS * 128], F32, tag=f"mm{ch}", name=f"mm{ch}_{band}")
              for ch in range(2)]
        for seg in range(SEGS):
            base = seg * SEGW
            for ch in range(2):
                nc.tensor.matmul(mm[ch][:, seg * 128:(seg + 1) * 128],
                                 ld[:, base + ch * 128:base + (ch + 1) * 128],
                                 ld[:, base + C:base + SEGW],
                                 start=True, stop=True)
        for ch in range(2):
            # psum free = seg*128 + Hr*16 + Wr ; stage free = Hr*128 + seg*16 + Wr
            # split each band's output into two 4-row half-bands so stores
            # start as soon as half the copies are done
            mm_v = mm[ch][:].rearrange("p (s hr wr) -> p s hr wr", s=SEGS, hr=8)
            for hb in range(2):
                stage = stage_pool.tile([128, SEGS * 64], F16, tag=f"st{ch}",
                                        name=f"st{ch}_{band}_{hb}")
                st_v = stage[:].rearrange("p (hr s wr) -> p s hr wr", s=SEGS, hr=4)
                if ch == 0:
                    nc.scalar.copy(st_v, mm_v[:, :, 4 * hb:4 * hb + 4, :])
                else:
                    nc.vector.tensor_copy(st_v, mm_v[:, :, 4 * hb:4 * hb + 4, :])
                st_eng = nc.sync if ch == 0 else nc.scalar
                st_eng.dma_start(out[ch * 128:(ch + 1) * 128,
                                     band * 8 + 4 * hb:band * 8 + 4 * hb + 4, :],
                                 stage[:])


def build_program():
    nc = bacc.Bacc("TRN2", target_bir_lowering=False, debug=False,
                   enable_asserts=False, num_devices=NCORES,
                   enable_partition_id=False)
    combo = nc.dram_tensor("combo", [BANDS, KP, SEGS * SEGW], F16,
                           kind="ExternalInput").ap()
    out = nc.dram_tensor("out", [C, HO // 2, WO], F16, kind="ExternalOutput").ap()
    with tile.TileContext(nc) as tc:
        with ExitStack() as ctx:
            _carafe_body(ctx, tc, out, combo)
    nc.compile()
    return nc


def make_in_maps(features: np.ndarray, masks: np.ndarray) -> list[dict]:
    features = np.asarray(features, dtype=np.float32)
    masks = np.asarray(masks, dtype=np.float32)
    feat_pad = np.pad(features, ((0, 0), (0, 0), (2, 2), (2, 2)))
    in_maps = []
    for core in range(NCORES):
        n, half = core // 2, core % 2
        fs = feat_pad[n, :, 32 * half:32 * half + 36, :]
        ms = masks[n, :, 64 * half:64 * half + 64, :]
        in_maps.append({"combo": _build_combo(fs, ms)})
    return in_maps


_CACHE: dict = {}


def _get_program():
    if "nc" not in _CACHE:
        _CACHE["nc"] = build_program()
    return _CACHE["nc"]


def kernel(features: np.ndarray, masks: np.ndarray) -> np.ndarray:
    in_maps = make_in_maps(features, masks)
    nc = _get_program()
    try:
        res = bass_utils.run_bass_kernel_spmd(nc, in_maps, core_ids=list(range(NCORES)))
    except Exception:
        # transient device errors (e.g. a wedged core from a prior run) usually
        # clear on retry
        res = bass_utils.run_bass_kernel_spmd(nc, in_maps, core_ids=list(range(NCORES)))
    out = np.empty((N, C, HO, WO), np.float32)
    for core in range(NCORES):
        n, half = core // 2, core % 2
        out[n, :, 64 * half:64 * half + 64, :] = res.results[core]["out"].astype(np.float32)
    return out
